# revision 8
# baseline (speedup 1.0000x reference)
"""Trainium2 Bass kernel for nn_NewAttention (B=4, S=2048, D=1024, H=16, DH=64).

Sharding: data-parallel over the 4 batches x tensor-parallel over 2 head-halves
(8 NeuronCores). Each core computes QKV projections + RoPE + causal attention
for its 8 heads of its batch, plus its partial output projection; the host sums
the two half partials per batch and transposes.

v2: QK^T in fp8e4m3 with DoubleRow perf mode (2x PE throughput), diagonal-
trimmed score matmuls/exp, triangular mask via DVE multiply instead of PE
matmul, softmax-normalization broadcast via DRAM-round-trip DMA instead of
PE matmul, rope arithmetic on the Pool engine in fp16, software-pipelined
instruction issue (next chunk's projections interleaved into this chunk's
attention stream so the in-order PE queue never starves).

Self-contained: builds/compiles the Bass program on first call and runs it on
cores 0-7 via concourse.bass_utils.run_bass_kernel_spmd.
"""

from contextlib import ExitStack
from dataclasses import dataclass

import numpy as np
import ml_dtypes

import concourse.bass as bass
import concourse.mybir as mybir
import concourse.tile as tile
from concourse.vector_clock import ScopedClock

# =========================================================================
# workarounds for this walrus build (sync-wait limits, missing NTFF glue)
# =========================================================================

MAX_CTRL_WAITS = 1


def _patched_drain_and_barrier(self, tick_clock, wait_clock):
    nop1 = self.nc.sync.nop(nofuse=True, hint="drain_waits")
    wait_clock.add_sem_waits(nop1.ins, ScopedClock({None: tick_clock.global_clock}))
    si = nop1.ins.sync_info
    if si is not None and si.on_wait and len(si.on_wait) > MAX_CTRL_WAITS:
        waits = list(si.on_wait)
        si.on_wait = waits[:MAX_CTRL_WAITS]
        rest = waits[MAX_CTRL_WAITS:]
        for i in range(0, len(rest), MAX_CTRL_WAITS):
            n = self.nc.sync.nop(nofuse=True, hint="drain_waits")
            chunk = rest[i : i + MAX_CTRL_WAITS]
            if n.ins.sync_info is None:
                import concourse.mybir as mybir

                n.ins.sync_info = mybir.SyncInfo(on_update=[], on_wait=chunk)
            else:
                n.ins.sync_info.on_wait.extend(chunk)

    self.nc.sync.drain()

    self.nc.all_engine_barrier()
    assert self.sems is not None
    popped = self.nc._tile_sem_poison_stack.pop()
    assert popped is self._sem_poison
    self.nc.clear_and_free_semaphores(list(self.sems.allocated().values()))
    self.nc.all_engine_barrier()


def fix_bir_sync_waits(bir: dict, max_waits: int = 1) -> int:
    """Split instructions carrying more than max_waits sync-waits: hoist the
    excess onto NoOps inserted just before, on the same engine queue."""
    ctr = 0
    for fn in bir.get("functions", []):
        for blk in fn.get("blocks", []):
            new = []
            for ins in blk.get("instructions", []):
                si = ins.get("sync_info") or {}
                waits = si.get("on_wait") or []
                if len(waits) > max_waits:
                    keep = waits[-max_waits:]
                    rest = waits[: len(waits) - max_waits]
                    for i in range(0, len(rest), max_waits):
                        ctr += 1
                        new.append(
                            {
                                "engine": ins["engine"],
                                "ins": [],
                                "outs": [],
                                "name": f"I-sw{ctr}",
                                "opcode": "NoOp",
                                "sync_info": {
                                    "on_update": [],
                                    "on_wait": rest[i : i + max_waits],
                                },
                                "text_hint": "split_waits",
                            }
                        )
                    si["on_wait"] = keep
                new.append(ins)
            blk["instructions"] = new
    return ctr


def _install_bir_fixup():
    import json

    import concourse.bass_utils as bass_utils
    import concourse.bass2jax as bass2jax

    orig = bass_utils.compile_bir_kernel
    if getattr(orig, "_sync_wait_fixup", False):
        return

    def patched(bir_json, tmpdir, neff_name="file.neff", **kw):
        bir = json.loads(bir_json)
        n = fix_bir_sync_waits(bir)
        if n:
            log_args = (f"tile_patch: split {n} excess sync-waits onto NoOps",)
            print(*log_args)
        return orig(json.dumps(bir).encode(), tmpdir, neff_name, **kw)

    patched._sync_wait_fixup = True
    bass_utils.compile_bir_kernel = patched
    bass2jax.compile_bir_kernel = patched


def apply():
    tile.TileContext._drain_and_barrier = _patched_drain_and_barrier
    _install_bir_fixup()
    _install_ntff_shim()


def _install_ntff_shim():
    """The agent image's antenv lacks axon_hooks; recreate the NTFF profile
    hook glue from trn_agent_boot so trace=True works under axon."""
    import sys
    import types

    try:
        from antenv.axon_hooks import get_axon_ntff_profile_hook  # noqa: F401
        return
    except ImportError:
        pass
    mod = types.ModuleType("antenv.axon_hooks")
    _hook = [None]
    mod.set_axon_ntff_profile_hook = lambda h: _hook.__setitem__(0, h)
    mod.get_axon_ntff_profile_hook = lambda: _hook[0]
    sys.modules["antenv.axon_hooks"] = mod
    import antenv

    antenv.axon_hooks = mod
    try:
        from trn_agent_boot.trn_boot import _ntff_profile_via_ctypes

        mod.set_axon_ntff_profile_hook(
            _ntff_profile_via_ctypes("/opt/axon/libaxon_pjrt.so"))
    except Exception:
        pass
    import concourse.bass_utils as bass_utils

    bass_utils.upload_artifacts = lambda tmpdir: tmpdir


# =========================================================================
# kernel builder
# =========================================================================

F32 = mybir.dt.float32
FP16 = mybir.dt.float16
FP8 = mybir.dt.float8e4
AF = mybir.ActivationFunctionType
PM = mybir.MatmulPerfMode


@dataclass
class Cfg:
    S: int = 2048      # sequence length
    D: int = 1024      # model dim
    DOUT: int = 512    # head dims on this core (H*64)
    CH: int = 512      # s-chunk size
    THETA: float = 10000.0

    @property
    def KT(self):      # contraction tiles over D
        return self.D // 128

    @property
    def P(self):       # head pairs (128-row groups of DOUT)
        return self.DOUT // 128

    @property
    def H(self):       # heads on this core
        return self.DOUT // 64

    @property
    def NCH(self):     # s-chunks
        return self.S // self.CH

    @property
    def CB(self):      # 128-col blocks per chunk
        return self.CH // 128

    @property
    def NT(self):      # total 128-t-tiles
        return self.S // 128


def _interleave(main_units, side_units):
    """Emit main_units in order, spreading side_units evenly between them."""
    si = 0
    n_side = len(side_units)
    n_main = max(1, len(main_units))
    for i, u in enumerate(main_units):
        u()
        want = n_side * (i + 1) // n_main
        while si < want:
            side_units[si]()
            si += 1
    while si < n_side:
        side_units[si]()
        si += 1


def build_nc(cfg: Cfg) -> bass.Bass:
    S, D, DOUT, CH = cfg.S, cfg.D, cfg.DOUT, cfg.CH
    KT, P, H, NCH, CB = cfg.KT, cfg.P, cfg.H, cfg.NCH, cfg.CB

    nc = bass.Bass("TRN2", target_bir_lowering=False)

    xT_d = nc.dram_tensor("xT", [D, S], FP16, kind="ExternalInput")
    wq_d = nc.dram_tensor("wq", [128, KT * DOUT], FP16, kind="ExternalInput")
    wk_d = nc.dram_tensor("wk", [128, KT * DOUT], FP16, kind="ExternalInput")
    wv_d = nc.dram_tensor("wv", [128, KT * DOUT], FP16, kind="ExternalInput")
    wo_d = nc.dram_tensor("wo", [128, P * D], FP16, kind="ExternalInput")
    cos_d = nc.dram_tensor("cos", [128, S], FP16, kind="ExternalInput")
    sin_d = nc.dram_tensor("sin", [128, S], FP16, kind="ExternalInput")
    msk_d = nc.dram_tensor("msk", [128, 128], FP16, kind="ExternalInput")
    outT_d = nc.dram_tensor("outT", [D, S], F32, kind="ExternalOutput")
    rcs_d = nc.dram_tensor("rcscr", [NCH * P, 2 * CH], FP16, kind="Internal")

    with tile.TileContext(nc) as tc, ExitStack() as ctx:
        ctx.enter_context(nc.allow_low_precision(reason="fp16/fp8 matmul operand production"))
        cons = ctx.enter_context(tc.tile_pool(name="cons", bufs=1))
        xtp = ctx.enter_context(tc.tile_pool(name="xt", bufs=16))
        rope = ctx.enter_context(tc.tile_pool(name="rope", bufs=2))
        q8p = ctx.enter_context(tc.tile_pool(name="q8p", bufs=2))
        exp = ctx.enter_context(tc.tile_pool(name="exp", bufs=3))
        outp = ctx.enter_context(tc.tile_pool(name="outc", bufs=2))
        smal = ctx.enter_context(tc.tile_pool(name="smal", bufs=2))
        bcp = ctx.enter_context(tc.tile_pool(name="bcp", bufs=2))
        psA = ctx.enter_context(tc.tile_pool(name="psA", bufs=2, space="PSUM"))
        psS = ctx.enter_context(tc.tile_pool(name="psS", bufs=2, space="PSUM"))
        psU = ctx.enter_context(tc.tile_pool(name="psU", bufs=2, space="PSUM"))

        # ---- resident constants / persistent tensors
        wq_s = cons.tile([128, KT * DOUT], FP16, tag="wq")
        nc.sync.dma_start(wq_s[:], wq_d[:])
        wk_s = cons.tile([128, KT * DOUT], FP16, tag="wk")
        nc.sync.dma_start(wk_s[:], wk_d[:])
        wv_s = cons.tile([128, KT * DOUT], FP16, tag="wv")
        nc.sync.dma_start(wv_s[:], wv_d[:])
        wo_s = cons.tile([128, P * D], FP16, tag="wo")
        nc.sync.dma_start(wo_s[:], wo_d[:])
        msk_s = cons.tile([128, 128], FP16, tag="msk")
        nc.sync.dma_start(msk_s[:], msk_d[:])
        cos_s = cons.tile([128, S], FP16, tag="cos")
        nc.sync.dma_start(cos_s[:], cos_d[:])
        sin_s = cons.tile([128, S], FP16, tag="sin")
        nc.sync.dma_start(sin_s[:], sin_d[:])

        hoTp = ctx.enter_context(tc.tile_pool(name="hoTp", bufs=2))
        hoT_cur = {}
        # packed fp8 q/k: per 2-head group g, rows (h%2)*32..+32, free [a*S + t]
        kt8 = [cons.tile([64, 2 * S], FP8, tag=f"kt8_{g}", name=f"kt8_{g}")
               for g in range(4)]
        qt8_cur = {}
        v_sb = cons.tile([128, cfg.NT * H * 65], FP16, tag="v_sb")
        v_ones = v_sb[:].rearrange("p (t g) -> p t g", g=65)[:, :, 64:65]
        nc.vector.memset(v_ones, 1.0)

        xt_cur = {}

        # ================= unit builders =================

        def proj_units(c):
            """Closures for chunk c's projections (x DMA, q/k proj+rope+fp8,
            v proj+copy)."""
            units = []

            def xt_dma():
                xs = []
                for kt in range(KT):
                    t = xtp.tile([128, CH], FP16, tag="xt")
                    nc.sync.dma_start(
                        t[:], xT_d[kt * 128 : (kt + 1) * 128, c * CH : (c + 1) * CH])
                    xs.append(t)
                xt_cur[c] = xs

            units.append(xt_dma)

            def qt8_alloc():
                qt8_cur[c] = [q8p.tile([64, 2 * CH], FP8, tag=f"qt8_{g}",
                                       name=f"qt8_{c}_{g}")
                              for g in range(4)]

            units.append(qt8_alloc)

            def mk_chain(w_s, p):
                def chain():
                    ps = psA.tile([128, CH], F32, tag="proj")
                    xt = xt_cur[c]
                    for kt in range(KT):
                        nc.tensor.matmul(
                            ps[:], w_s[:, kt * DOUT + p * 128 : kt * DOUT + (p + 1) * 128],
                            xt[kt][:], start=(kt == 0), stop=(kt == KT - 1))
                    chain.ps = ps
                return chain

            def mk_finish(chain, p, is_q):
                def finish():
                    ps = chain.ps
                    cos_c = cos_s[:, c * CH : (c + 1) * CH]
                    sin_c = sin_s[:, c * CH : (c + 1) * CH]
                    q0 = rope.tile([128, CH], FP16, tag="q0")
                    nc.vector.tensor_copy(q0[:], ps[:])
                    sw = rope.tile([128, CH], FP16, tag="qsw")
                    for dst, src in ((0, 32), (32, 0), (64, 96), (96, 64)):
                        nc.sync.dma_start(sw[dst : dst + 32, :], q0[src : src + 32, :])
                    nc.gpsimd.tensor_mul(q0[:], q0[:], cos_c)
                    nc.gpsimd.tensor_mul(sw[:], sw[:], sin_c)
                    r8 = rope.tile([128, CH], FP8, tag="r8")
                    nc.gpsimd.tensor_add(r8[:], q0[:], sw[:])
                    # repack to [32, (a 2), t] per head
                    for h2 in range(2):
                        h = 2 * p + h2
                        g, r0 = h // 2, (h % 2) * 32
                        for a in range(2):
                            src_ap = r8[h2 * 64 + a * 32 : h2 * 64 + a * 32 + 32, :]
                            if is_q:
                                nc.sync.dma_start(
                                    qt8_cur[c][g][r0 : r0 + 32, a * CH : (a + 1) * CH],
                                    src_ap)
                            else:
                                nc.sync.dma_start(
                                    kt8[g][r0 : r0 + 32,
                                           a * S + c * CH : a * S + (c + 1) * CH],
                                    src_ap)
                return finish

            for p in range(P):
                ch_q = mk_chain(wq_s, p)
                units.append(ch_q)
                units.append(mk_finish(ch_q, p, True))
                ch_k = mk_chain(wk_s, p)
                units.append(ch_k)
                units.append(mk_finish(ch_k, p, False))

            def mk_v(st):
                def vproj():
                    ps = psA.tile([128, DOUT], F32, tag="proj")
                    xt = xt_cur[c]
                    for kt in range(KT):
                        nc.tensor.matmul(
                            ps[:], xt[kt][:, st * 128 : (st + 1) * 128],
                            wv_s[:, kt * DOUT : (kt + 1) * DOUT],
                            start=(kt == 0), stop=(kt == KT - 1))
                    stg = c * CB + st
                    dst = (v_sb[:, stg * H * 65 : (stg + 1) * H * 65]
                           .rearrange("p (h g) -> p h g", g=65)[:, :, 0:64])
                    nc.vector.tensor_copy(dst, ps[:].rearrange("p (h g) -> p h g", g=64))
                return vproj

            for st in range(CB):
                units.append(mk_v(st))
            return units

        def attn_units(c):
            """Closures for chunk c's attention: per pair, QK8+exp+mask / PV
            per t-tile, then normalization."""
            ntt = (c + 1) * CB
            units = []
            ucur = {}

            def mk_qk(p, tt):
                j = tt - c * CB
                diag = j >= 0
                ofs = j * 128 if diag else 0

                def qk():
                    if tt == 0:
                        ucur[p] = [psU.tile([65, CH], F32, tag="pu", name=f"u{h}")
                                   for h in range(2)]
                    ps = psS.tile([128, 2 * CH], F32, tag="ps_pair")
                    for h2 in range(2):
                        h = 2 * p + h2
                        g, r0 = h // 2, (h % 2) * 32
                        lhsT = (kt8[g][r0 : r0 + 32, :]
                                .rearrange("p (a t) -> p a t", a=2)
                                [:, :, tt * 128 : (tt + 1) * 128])
                        s0 = ofs
                        while s0 < CH:
                            n = min(256, CH - s0)
                            rhs = (qt8_cur[c][g][r0 : r0 + 32, :]
                                   .rearrange("p (a s) -> p a s", a=2)[:, :, s0 : s0 + n])
                            nc.tensor.matmul(
                                ps[:, h2 * CH + s0 : h2 * CH + s0 + n], lhsT, rhs,
                                start=True, stop=True, perf_mode=PM.DoubleRow,
                                skip_group_check=True)
                            s0 += n
                    ex = exp.tile([128, 2 * CH], FP16, tag="ex")
                    if diag:
                        nc.scalar.activation(
                            ex[:].rearrange("p (h n) -> p h n", h=2)[:, :, ofs:],
                            ps[:].rearrange("p (h n) -> p h n", h=2)[:, :, ofs:],
                            AF.Exp, scale=0.125)
                        for h2 in range(2):
                            sl = ex[:, h2 * CH + ofs : h2 * CH + ofs + 128]
                            nc.vector.tensor_mul(sl, sl, msk_s[:])
                    else:
                        nc.scalar.activation(ex[:], ps[:], AF.Exp, scale=0.125)
                    qk.ex = ex
                return qk

            def mk_pv(qk_unit, p, tt):
                j = tt - c * CB
                ofs = j * 128 if j >= 0 else 0

                def pv():
                    ex = qk_unit.ex
                    u = ucur[p]
                    for h2 in range(2):
                        nc.tensor.matmul(
                            u[h2][:, ofs:CH],
                            v_sb[:, (tt * H + p * 2 + h2) * 65 : (tt * H + p * 2 + h2) * 65 + 65],
                            ex[:, h2 * CH + ofs : (h2 + 1) * CH],
                            start=(tt == 0), stop=(tt == ntt - 1),
                            skip_group_check=True)
                return pv

            def mk_norm(p):
                def norm():
                    u = ucur[p]
                    sums = smal.tile([1, 2 * CH], F32, tag="sums")
                    for h2 in range(2):
                        nc.vector.tensor_copy(
                            sums[:, h2 * CH : (h2 + 1) * CH], u[h2][64:65, :])
                    s64 = smal.tile([64, 2 * CH // 64], F32, tag="s64")
                    nc.sync.dma_start(s64[:], sums[:])
                    r64 = smal.tile([64, 2 * CH // 64], FP16, tag="r64")
                    nc.vector.reciprocal(r64[:], s64[:])
                    rc = smal.tile([1, 2 * CH], FP16, tag="rc")
                    nc.sync.dma_start(rc[:], r64[:])
                    row = c * P + p
                    nc.sync.dma_start(rcs_d[row : row + 1, :], rc[:])
                    bc = bcp.tile([64, 2 * CH], FP16, tag="bc")
                    nc.sync.dma_start(
                        bc[:],
                        rcs_d[row : row + 1, :].partition_broadcast(64).squeeze(1))
                    ho = hoTp.tile([128, CH], FP16, tag=f"hoT{p}")
                    hoT_cur[(c, p)] = ho
                    for h2 in range(2):
                        nc.vector.tensor_mul(
                            ho[h2 * 64 : (h2 + 1) * 64, :],
                            u[h2][0:64, :], bc[:, h2 * CH : (h2 + 1) * CH])
                return norm

            for p in range(P):
                for tt in range(ntt):
                    qku = mk_qk(p, tt)
                    units.append(qku)
                    units.append(mk_pv(qku, p, tt))
                units.append(mk_norm(p))
            return units

        def outproj_units(c):
            units = []

            def mk_out(dt):
                def outproj():
                    ps_o = psA.tile([128, CH], F32, tag="proj")
                    for p in range(P):
                        nc.tensor.matmul(
                            ps_o[:], wo_s[:, p * D + dt * 128 : p * D + (dt + 1) * 128],
                            hoT_cur[(c, p)][:], start=(p == 0), stop=(p == P - 1))
                    oc = outp.tile([128, CH], F32, tag="oc")
                    if dt % 2 == 0:
                        nc.scalar.copy(oc[:], ps_o[:])
                    else:
                        nc.vector.tensor_copy(oc[:], ps_o[:])
                    nc.sync.dma_start(
                        outT_d[dt * 128 : (dt + 1) * 128, c * CH : (c + 1) * CH], oc[:])
                return outproj

            for dt in range(D // 128):
                units.append(mk_out(dt))
            return units

        # ================= schedule =================
        for u in proj_units(0):
            u()
        for c in range(NCH):
            side = []
            if c + 1 < NCH:
                side += proj_units(c + 1)
            if c - 1 >= 0:
                side += outproj_units(c - 1)
            _interleave(attn_units(c), side)
        for u in outproj_units(NCH - 1):
            u()

    return nc


# ---------------------------------------------------------------------------
# host-side input prep
# ---------------------------------------------------------------------------

def rope_tables(S, DH, theta):
    freqs = 1.0 / (theta ** (np.arange(0, DH, 2, dtype=np.float32) / DH))
    ang = np.outer(np.arange(S, dtype=np.float32), freqs)  # [S, DH//2]
    return np.cos(ang).astype(np.float32), np.sin(ang).astype(np.float32)


def grouped_perm(DH=64):
    """within-head dim permutation: [0,2,..62, 1,3,..63]"""
    return np.concatenate([np.arange(0, DH, 2), np.arange(1, DH, 2)])


def prep_core_inputs(cfg: Cfg, x_b, Wq_h, Wk_h, Wv_h, Wo_cols):
    """x_b [S, D]; Wq_h/Wk_h/Wv_h [DOUT, D] (this half's rows);
    Wo_cols [D, DOUT] (this half's columns of Wo)."""
    S, D, DOUT, KT, P, H = cfg.S, cfg.D, cfg.DOUT, cfg.KT, cfg.P, cfg.H
    DH = 64
    perm = grouped_perm(DH)
    # permute q/k output dims to grouped-even/odd within each head
    gperm = np.concatenate([h * DH + perm for h in range(H)])

    def wtile(Wt):  # [DOUT, D] -> [128, KT*DOUT] k-tile-major of W.T
        wt = np.ascontiguousarray(Wt.T)  # [D, DOUT]
        return np.ascontiguousarray(
            wt.reshape(KT, 128, DOUT).transpose(1, 0, 2).reshape(128, KT * DOUT))

    # scale 1/sqrt(DH) now applied inside the exp activation (scale=0.125)
    wq = wtile(Wq_h[gperm]).astype(np.float16)
    wk = wtile(Wk_h[gperm]).astype(np.float16)
    wv = wtile(Wv_h).astype(np.float16)
    wo_t = np.ascontiguousarray(Wo_cols.T)  # [DOUT, D]
    wo = np.ascontiguousarray(
        wo_t.reshape(P, 128, D).transpose(1, 0, 2).reshape(128, P * D)).astype(np.float16)

    cos_t, sin_t = rope_tables(S, DH, cfg.THETA)  # [S, 32]
    # grouped rows: r in [0,128): head=r//64, pos=r%64: i=pos%32, sign=-1 if pos<32 else +1
    pos = np.arange(128) % 64
    i = pos % 32
    sign = np.where(pos < 32, -1.0, 1.0).astype(np.float32)
    cos_g = np.ascontiguousarray(cos_t.T[i]).astype(np.float16)       # [128, S]
    sin_g = np.ascontiguousarray(sin_t.T[i] * sign[:, None]).astype(np.float16)

    r = np.arange(128)
    msk01 = np.where(r[None, :] >= r[:, None], 1.0, 0.0).astype(np.float16)

    return {
        "xT": np.ascontiguousarray(x_b.T).astype(np.float16),
        "wq": wq, "wk": wk, "wv": wv, "wo": wo,
        "cos": cos_g, "sin": sin_g,
        "msk": msk01,
    }


# =========================================================================
# public entry point
# =========================================================================

_CACHE = {}


def kernel(x, Wq, Wk, Wv, Wo, lambdas=None, trace=False):
    from concourse.bass_utils import run_bass_kernel_spmd

    if not _CACHE.get("patched"):
        apply()
        _CACHE["patched"] = True
    x = np.asarray(x, dtype=np.float32)
    Wq = np.asarray(Wq, dtype=np.float32)
    Wk = np.asarray(Wk, dtype=np.float32)
    Wv = np.asarray(Wv, dtype=np.float32)
    Wo = np.asarray(Wo, dtype=np.float32)
    cfg = Cfg()
    if "nc" not in _CACHE:
        _CACHE["nc"] = build_nc(cfg)
    nc = _CACHE["nc"]
    in_maps = []
    for core in range(8):
        b, half = core // 2, core % 2
        sl = slice(half * cfg.DOUT, (half + 1) * cfg.DOUT)
        in_maps.append(prep_core_inputs(cfg, x[b], Wq[sl], Wk[sl], Wv[sl], Wo[:, sl]))
    res = run_bass_kernel_spmd(nc, in_maps, list(range(8)), trace=trace)
    outs = res.results
    out = np.stack(
        [(outs[2 * b]["outT"] + outs[2 * b + 1]["outT"]).T for b in range(4)]
    ).astype(np.float32)
    if trace:
        return out, res
    return out


# revision 10
# speedup vs baseline: 1.2847x; 1.2847x over previous
"""Trainium2 Bass kernel for nn_NewAttention (B=4, S=2048, D=1024, H=16, DH=64).

Sharding: data-parallel over the 4 batches x tensor-parallel over 2 head-halves
(8 NeuronCores). Each core computes QKV projections + RoPE + causal attention
for its 8 heads of its batch, plus its partial output projection; the host sums
the two half partials per batch and transposes.

v2: QK^T in fp8e4m3 with DoubleRow perf mode (2x PE throughput), diagonal-
trimmed score matmuls/exp, triangular mask via DVE multiply instead of PE
matmul, softmax-normalization broadcast via DRAM-round-trip DMA instead of
PE matmul, rope arithmetic on the Pool engine in fp16, software-pipelined
instruction issue (next chunk's projections interleaved into this chunk's
attention stream so the in-order PE queue never starves).

Self-contained: builds/compiles the Bass program on first call and runs it on
cores 0-7 via concourse.bass_utils.run_bass_kernel_spmd.
"""

from contextlib import ExitStack
from dataclasses import dataclass

import numpy as np
import ml_dtypes

import concourse.bass as bass
import concourse.mybir as mybir
import concourse.tile as tile
from concourse.vector_clock import ScopedClock

# =========================================================================
# workarounds for this walrus build (sync-wait limits, missing NTFF glue)
# =========================================================================

MAX_CTRL_WAITS = 1


def _patched_drain_and_barrier(self, tick_clock, wait_clock):
    nop1 = self.nc.sync.nop(nofuse=True, hint="drain_waits")
    wait_clock.add_sem_waits(nop1.ins, ScopedClock({None: tick_clock.global_clock}))
    si = nop1.ins.sync_info
    if si is not None and si.on_wait and len(si.on_wait) > MAX_CTRL_WAITS:
        waits = list(si.on_wait)
        si.on_wait = waits[:MAX_CTRL_WAITS]
        rest = waits[MAX_CTRL_WAITS:]
        for i in range(0, len(rest), MAX_CTRL_WAITS):
            n = self.nc.sync.nop(nofuse=True, hint="drain_waits")
            chunk = rest[i : i + MAX_CTRL_WAITS]
            if n.ins.sync_info is None:
                import concourse.mybir as mybir

                n.ins.sync_info = mybir.SyncInfo(on_update=[], on_wait=chunk)
            else:
                n.ins.sync_info.on_wait.extend(chunk)

    self.nc.sync.drain()

    self.nc.all_engine_barrier()
    assert self.sems is not None
    popped = self.nc._tile_sem_poison_stack.pop()
    assert popped is self._sem_poison
    self.nc.clear_and_free_semaphores(list(self.sems.allocated().values()))
    self.nc.all_engine_barrier()


def fix_bir_sync_waits(bir: dict, max_waits: int = 1) -> int:
    """Split instructions carrying more than max_waits sync-waits: hoist the
    excess onto NoOps inserted just before, on the same engine queue."""
    ctr = 0
    for fn in bir.get("functions", []):
        for blk in fn.get("blocks", []):
            new = []
            for ins in blk.get("instructions", []):
                si = ins.get("sync_info") or {}
                waits = si.get("on_wait") or []
                if len(waits) > max_waits:
                    keep = waits[-max_waits:]
                    rest = waits[: len(waits) - max_waits]
                    for i in range(0, len(rest), max_waits):
                        ctr += 1
                        new.append(
                            {
                                "engine": ins["engine"],
                                "ins": [],
                                "outs": [],
                                "name": f"I-sw{ctr}",
                                "opcode": "NoOp",
                                "sync_info": {
                                    "on_update": [],
                                    "on_wait": rest[i : i + max_waits],
                                },
                                "text_hint": "split_waits",
                            }
                        )
                    si["on_wait"] = keep
                new.append(ins)
            blk["instructions"] = new
    return ctr


def _install_bir_fixup():
    import json

    import concourse.bass_utils as bass_utils
    import concourse.bass2jax as bass2jax

    orig = bass_utils.compile_bir_kernel
    if getattr(orig, "_sync_wait_fixup", False):
        return

    def patched(bir_json, tmpdir, neff_name="file.neff", **kw):
        bir = json.loads(bir_json)
        n = fix_bir_sync_waits(bir)
        if n:
            log_args = (f"tile_patch: split {n} excess sync-waits onto NoOps",)
            print(*log_args)
        return orig(json.dumps(bir).encode(), tmpdir, neff_name, **kw)

    patched._sync_wait_fixup = True
    bass_utils.compile_bir_kernel = patched
    bass2jax.compile_bir_kernel = patched


def apply():
    tile.TileContext._drain_and_barrier = _patched_drain_and_barrier
    _install_bir_fixup()
    _install_ntff_shim()


def _install_ntff_shim():
    """The agent image's antenv lacks axon_hooks; recreate the NTFF profile
    hook glue from trn_agent_boot so trace=True works under axon."""
    import sys
    import types

    try:
        from antenv.axon_hooks import get_axon_ntff_profile_hook  # noqa: F401
        return
    except ImportError:
        pass
    mod = types.ModuleType("antenv.axon_hooks")
    _hook = [None]
    mod.set_axon_ntff_profile_hook = lambda h: _hook.__setitem__(0, h)
    mod.get_axon_ntff_profile_hook = lambda: _hook[0]
    sys.modules["antenv.axon_hooks"] = mod
    import antenv

    antenv.axon_hooks = mod
    try:
        from trn_agent_boot.trn_boot import _ntff_profile_via_ctypes

        mod.set_axon_ntff_profile_hook(
            _ntff_profile_via_ctypes("/opt/axon/libaxon_pjrt.so"))
    except Exception:
        pass
    import concourse.bass_utils as bass_utils

    bass_utils.upload_artifacts = lambda tmpdir: tmpdir


# =========================================================================
# kernel builder
# =========================================================================

F32 = mybir.dt.float32
FP16 = mybir.dt.float16
FP8 = mybir.dt.float8e4
AF = mybir.ActivationFunctionType
PM = mybir.MatmulPerfMode


@dataclass
class Cfg:
    S: int = 2048      # sequence length
    D: int = 1024      # model dim
    DOUT: int = 512    # head dims on this core (H*64)
    CH: int = 512      # s-chunk size
    THETA: float = 10000.0

    @property
    def KT(self):      # contraction tiles over D
        return self.D // 128

    @property
    def P(self):       # head pairs (128-row groups of DOUT)
        return self.DOUT // 128

    @property
    def H(self):       # heads on this core
        return self.DOUT // 64

    @property
    def NCH(self):     # s-chunks
        return self.S // self.CH

    @property
    def CB(self):      # 128-col blocks per chunk
        return self.CH // 128

    @property
    def NT(self):      # total 128-t-tiles
        return self.S // 128


def _interleave(main_units, side_units):
    """Emit main_units in order, spreading side_units evenly between them."""
    si = 0
    n_side = len(side_units)
    n_main = max(1, len(main_units))
    for i, u in enumerate(main_units):
        u()
        want = n_side * (i + 1) // n_main
        while si < want:
            side_units[si]()
            si += 1
    while si < n_side:
        side_units[si]()
        si += 1


def build_nc(cfg: Cfg) -> bass.Bass:
    S, D, DOUT, CH = cfg.S, cfg.D, cfg.DOUT, cfg.CH
    KT, P, H, NCH, CB = cfg.KT, cfg.P, cfg.H, cfg.NCH, cfg.CB

    nc = bass.Bass("TRN2", target_bir_lowering=False)

    xT_d = nc.dram_tensor("xT", [D, S], FP16, kind="ExternalInput")
    wq_d = nc.dram_tensor("wq", [128, KT * DOUT], FP16, kind="ExternalInput")
    wk_d = nc.dram_tensor("wk", [128, KT * DOUT], FP16, kind="ExternalInput")
    wv_d = nc.dram_tensor("wv", [128, KT * DOUT], FP16, kind="ExternalInput")
    wo_d = nc.dram_tensor("wo", [128, P * D], FP16, kind="ExternalInput")
    cos_d = nc.dram_tensor("cos", [128, S], FP16, kind="ExternalInput")
    sin_d = nc.dram_tensor("sin", [128, S], FP16, kind="ExternalInput")
    msk_d = nc.dram_tensor("msk", [128, 128], FP16, kind="ExternalInput")
    outT_d = nc.dram_tensor("outT", [D, S], F32, kind="ExternalOutput")
    rcs_d = nc.dram_tensor("rcscr", [NCH * P, 2 * CH], FP16, kind="Internal")

    with tile.TileContext(nc) as tc, ExitStack() as ctx:
        ctx.enter_context(nc.allow_low_precision(reason="fp16/fp8 matmul operand production"))
        cons = ctx.enter_context(tc.tile_pool(name="cons", bufs=1))
        xtp = ctx.enter_context(tc.tile_pool(name="xt", bufs=16))
        rope = ctx.enter_context(tc.tile_pool(name="rope", bufs=2))
        q8p = ctx.enter_context(tc.tile_pool(name="q8p", bufs=2))
        exp = ctx.enter_context(tc.tile_pool(name="exp", bufs=3))
        outp = ctx.enter_context(tc.tile_pool(name="outc", bufs=2))
        smal = ctx.enter_context(tc.tile_pool(name="smal", bufs=2))
        bcp = ctx.enter_context(tc.tile_pool(name="bcp", bufs=2))
        psA = ctx.enter_context(tc.tile_pool(name="psA", bufs=2, space="PSUM"))
        psS = ctx.enter_context(tc.tile_pool(name="psS", bufs=2, space="PSUM"))
        psU = ctx.enter_context(tc.tile_pool(name="psU", bufs=2, space="PSUM"))

        # ---- resident constants / persistent tensors
        wq_s = cons.tile([128, KT * DOUT], FP16, tag="wq")
        nc.sync.dma_start(wq_s[:], wq_d[:])
        wk_s = cons.tile([128, KT * DOUT], FP16, tag="wk")
        nc.sync.dma_start(wk_s[:], wk_d[:])
        wv_s = cons.tile([128, KT * DOUT], FP16, tag="wv")
        nc.sync.dma_start(wv_s[:], wv_d[:])
        wo_s = cons.tile([128, P * D], FP16, tag="wo")
        nc.sync.dma_start(wo_s[:], wo_d[:])
        msk_s = cons.tile([128, 128], FP16, tag="msk")
        nc.sync.dma_start(msk_s[:], msk_d[:])
        cos_s = cons.tile([128, S], FP16, tag="cos")
        nc.sync.dma_start(cos_s[:], cos_d[:])
        sin_s = cons.tile([128, S], FP16, tag="sin")
        nc.sync.dma_start(sin_s[:], sin_d[:])

        hoTp = ctx.enter_context(tc.tile_pool(name="hoTp", bufs=2))
        hoT_cur = {}
        # packed fp8 q/k: per 2-head group g, rows (h%2)*32..+32, free [a*S + t]
        kt8 = [cons.tile([64, 2 * S], FP8, tag=f"kt8_{g}", name=f"kt8_{g}")
               for g in range(4)]
        qt8_cur = {}
        v_sb = cons.tile([128, cfg.NT * H * 65], FP16, tag="v_sb")
        v_ones = v_sb[:].rearrange("p (t g) -> p t g", g=65)[:, :, 64:65]
        nc.vector.memset(v_ones, 1.0)

        xt_cur = {}

        # ================= unit builders =================

        def proj_units(c):
            """Closures for chunk c's projections (x DMA, q/k proj+rope+fp8,
            v proj+copy)."""
            units = []

            def xt_dma():
                xs = []
                for kt in range(KT):
                    t = xtp.tile([128, CH], FP16, tag="xt")
                    nc.sync.dma_start(
                        t[:], xT_d[kt * 128 : (kt + 1) * 128, c * CH : (c + 1) * CH])
                    xs.append(t)
                xt_cur[c] = xs

            units.append(xt_dma)

            def qt8_alloc():
                qt8_cur[c] = [q8p.tile([64, 2 * CH], FP8, tag=f"qt8_{g}",
                                       name=f"qt8_{c}_{g}")
                              for g in range(4)]

            units.append(qt8_alloc)

            def mk_chain(w_s, p):
                def chain():
                    ps = psA.tile([128, CH], F32, tag="proj")
                    xt = xt_cur[c]
                    for kt in range(KT):
                        nc.tensor.matmul(
                            ps[:], w_s[:, kt * DOUT + p * 128 : kt * DOUT + (p + 1) * 128],
                            xt[kt][:], start=(kt == 0), stop=(kt == KT - 1))
                    chain.ps = ps
                return chain

            def mk_finish(chain, p, is_q):
                def finish():
                    ps = chain.ps
                    cos_c = cos_s[:, c * CH : (c + 1) * CH]
                    sin_c = sin_s[:, c * CH : (c + 1) * CH]
                    q0 = rope.tile([128, CH], FP16, tag="q0")
                    nc.vector.tensor_copy(q0[:], ps[:])
                    sw = rope.tile([128, CH], FP16, tag="qsw")
                    for dst, src in ((0, 32), (32, 0), (64, 96), (96, 64)):
                        nc.sync.dma_start(sw[dst : dst + 32, :], q0[src : src + 32, :])
                    nc.vector.tensor_mul(q0[:], q0[:], cos_c)
                    nc.vector.tensor_mul(sw[:], sw[:], sin_c)
                    r8 = rope.tile([128, CH], FP8, tag="r8")
                    nc.gpsimd.tensor_add(r8[:], q0[:], sw[:])
                    # repack to [32, (a 2), t] per head
                    for h2 in range(2):
                        h = 2 * p + h2
                        g, r0 = h // 2, (h % 2) * 32
                        for a in range(2):
                            src_ap = r8[h2 * 64 + a * 32 : h2 * 64 + a * 32 + 32, :]
                            if is_q:
                                nc.sync.dma_start(
                                    qt8_cur[c][g][r0 : r0 + 32, a * CH : (a + 1) * CH],
                                    src_ap)
                            else:
                                nc.sync.dma_start(
                                    kt8[g][r0 : r0 + 32,
                                           a * S + c * CH : a * S + (c + 1) * CH],
                                    src_ap)
                return finish

            for p in range(P):
                ch_q = mk_chain(wq_s, p)
                units.append(ch_q)
                units.append(mk_finish(ch_q, p, True))
                ch_k = mk_chain(wk_s, p)
                units.append(ch_k)
                units.append(mk_finish(ch_k, p, False))

            def mk_v(st):
                def vproj():
                    ps = psA.tile([128, DOUT], F32, tag="proj")
                    xt = xt_cur[c]
                    for kt in range(KT):
                        nc.tensor.matmul(
                            ps[:], xt[kt][:, st * 128 : (st + 1) * 128],
                            wv_s[:, kt * DOUT : (kt + 1) * DOUT],
                            start=(kt == 0), stop=(kt == KT - 1))
                    stg = c * CB + st
                    dst = (v_sb[:, stg * H * 65 : (stg + 1) * H * 65]
                           .rearrange("p (h g) -> p h g", g=65)[:, :, 0:64])
                    nc.vector.tensor_copy(dst, ps[:].rearrange("p (h g) -> p h g", g=64))
                return vproj

            for st in range(CB):
                units.append(mk_v(st))
            return units

        def attn_units(c):
            """Closures for chunk c's attention: per pair, QK8+exp+mask / PV
            per t-tile, then normalization."""
            ntt = (c + 1) * CB
            units = []
            ucur = {}

            def mk_qk(p, tt):
                j = tt - c * CB
                diag = j >= 0
                ofs = j * 128 if diag else 0

                def qk():
                    if tt == 0:
                        ucur[p] = [psU.tile([65, CH], F32, tag="pu", name=f"u{h}")
                                   for h in range(2)]
                    ps = psS.tile([128, 2 * CH], F32, tag="ps_pair")
                    for h2 in range(2):
                        h = 2 * p + h2
                        g, r0 = h // 2, (h % 2) * 32
                        lhsT = (kt8[g][r0 : r0 + 32, :]
                                .rearrange("p (a t) -> p a t", a=2)
                                [:, :, tt * 128 : (tt + 1) * 128])
                        s0 = ofs
                        while s0 < CH:
                            n = min(256, CH - s0)
                            rhs = (qt8_cur[c][g][r0 : r0 + 32, :]
                                   .rearrange("p (a s) -> p a s", a=2)[:, :, s0 : s0 + n])
                            nc.tensor.matmul(
                                ps[:, h2 * CH + s0 : h2 * CH + s0 + n], lhsT, rhs,
                                start=True, stop=True, perf_mode=PM.DoubleRow,
                                skip_group_check=True)
                            s0 += n
                    ex = exp.tile([128, 2 * CH], FP16, tag="ex")
                    if diag:
                        nc.scalar.activation(
                            ex[:].rearrange("p (h n) -> p h n", h=2)[:, :, ofs:],
                            ps[:].rearrange("p (h n) -> p h n", h=2)[:, :, ofs:],
                            AF.Exp, scale=0.125)
                        for h2 in range(2):
                            sl = ex[:, h2 * CH + ofs : h2 * CH + ofs + 128]
                            nc.vector.tensor_mul(sl, sl, msk_s[:])
                    else:
                        nc.scalar.activation(ex[:], ps[:], AF.Exp, scale=0.125)
                    qk.ex = ex
                return qk

            def mk_pv(qk_unit, p, tt):
                j = tt - c * CB
                ofs = j * 128 if j >= 0 else 0

                def pv():
                    ex = qk_unit.ex
                    u = ucur[p]
                    for h2 in range(2):
                        nc.tensor.matmul(
                            u[h2][:, ofs:CH],
                            v_sb[:, (tt * H + p * 2 + h2) * 65 : (tt * H + p * 2 + h2) * 65 + 65],
                            ex[:, h2 * CH + ofs : (h2 + 1) * CH],
                            start=(tt == 0), stop=(tt == ntt - 1),
                            skip_group_check=True)
                return pv

            def mk_norm(p):
                def norm():
                    u = ucur[p]
                    sums = smal.tile([1, 2 * CH], F32, tag="sums")
                    ho = hoTp.tile([128, CH], FP16, tag=f"hoT{p}")
                    hoT_cur[(c, p)] = ho
                    # drain u out of PSUM promptly so the next pair's QK can
                    # reuse the banks without waiting on the DMA round trip
                    for h2 in range(2):
                        nc.vector.tensor_copy(
                            sums[:, h2 * CH : (h2 + 1) * CH], u[h2][64:65, :])
                        nc.vector.tensor_copy(
                            ho[h2 * 64 : (h2 + 1) * 64, :], u[h2][0:64, :])
                    s64 = smal.tile([64, 2 * CH // 64], F32, tag="s64")
                    nc.sync.dma_start(s64[:], sums[:])
                    r64 = smal.tile([64, 2 * CH // 64], FP16, tag="r64")
                    nc.vector.reciprocal(r64[:], s64[:])
                    rc = smal.tile([1, 2 * CH], FP16, tag="rc")
                    nc.sync.dma_start(rc[:], r64[:])
                    row = c * P + p
                    nc.sync.dma_start(rcs_d[row : row + 1, :], rc[:])
                    bc = bcp.tile([128, 2 * CH], FP16, tag="bc")
                    nc.sync.dma_start(
                        bc[:],
                        rcs_d[row : row + 1, :].partition_broadcast(128).squeeze(1))
                    for h2 in range(2):
                        sl = ho[h2 * 64 : (h2 + 1) * 64, :]
                        nc.vector.tensor_mul(
                            sl, sl,
                            bc[h2 * 64 : (h2 + 1) * 64, h2 * CH : (h2 + 1) * CH])
                return norm

            pend = []  # (pv_unit, norm_unit_or_None) lagging one step
            for p in range(P):
                for tt in range(ntt):
                    qku = mk_qk(p, tt)
                    units.append(qku)
                    pend.append((mk_pv(qku, p, tt),
                                 mk_norm(p) if tt == ntt - 1 else None))
                    if len(pend) > 1:
                        pv_u, norm_u = pend.pop(0)
                        units.append(pv_u)
                        if norm_u is not None:
                            units.append(norm_u)
            while pend:
                pv_u, norm_u = pend.pop(0)
                units.append(pv_u)
                if norm_u is not None:
                    units.append(norm_u)
            return units

        def outproj_units(c):
            units = []

            def mk_out(dt):
                def outproj():
                    ps_o = psA.tile([128, CH], F32, tag="proj")
                    for p in range(P):
                        nc.tensor.matmul(
                            ps_o[:], wo_s[:, p * D + dt * 128 : p * D + (dt + 1) * 128],
                            hoT_cur[(c, p)][:], start=(p == 0), stop=(p == P - 1))
                    oc = outp.tile([128, CH], F32, tag="oc")
                    nc.vector.tensor_copy(oc[:], ps_o[:])
                    nc.sync.dma_start(
                        outT_d[dt * 128 : (dt + 1) * 128, c * CH : (c + 1) * CH], oc[:])
                return outproj

            for dt in range(D // 128):
                units.append(mk_out(dt))
            return units

        # ================= schedule =================
        for u in proj_units(0):
            u()
        for c in range(NCH):
            side = []
            if c + 1 < NCH:
                side += proj_units(c + 1)
            if c - 1 >= 0:
                side += outproj_units(c - 1)
            _interleave(attn_units(c), side)
        for u in outproj_units(NCH - 1):
            u()

    return nc


# ---------------------------------------------------------------------------
# host-side input prep
# ---------------------------------------------------------------------------

def rope_tables(S, DH, theta):
    freqs = 1.0 / (theta ** (np.arange(0, DH, 2, dtype=np.float32) / DH))
    ang = np.outer(np.arange(S, dtype=np.float32), freqs)  # [S, DH//2]
    return np.cos(ang).astype(np.float32), np.sin(ang).astype(np.float32)


def grouped_perm(DH=64):
    """within-head dim permutation: [0,2,..62, 1,3,..63]"""
    return np.concatenate([np.arange(0, DH, 2), np.arange(1, DH, 2)])


def prep_core_inputs(cfg: Cfg, x_b, Wq_h, Wk_h, Wv_h, Wo_cols):
    """x_b [S, D]; Wq_h/Wk_h/Wv_h [DOUT, D] (this half's rows);
    Wo_cols [D, DOUT] (this half's columns of Wo)."""
    S, D, DOUT, KT, P, H = cfg.S, cfg.D, cfg.DOUT, cfg.KT, cfg.P, cfg.H
    DH = 64
    perm = grouped_perm(DH)
    # permute q/k output dims to grouped-even/odd within each head
    gperm = np.concatenate([h * DH + perm for h in range(H)])

    def wtile(Wt):  # [DOUT, D] -> [128, KT*DOUT] k-tile-major of W.T
        wt = np.ascontiguousarray(Wt.T)  # [D, DOUT]
        return np.ascontiguousarray(
            wt.reshape(KT, 128, DOUT).transpose(1, 0, 2).reshape(128, KT * DOUT))

    # scale 1/sqrt(DH) now applied inside the exp activation (scale=0.125)
    wq = wtile(Wq_h[gperm]).astype(np.float16)
    wk = wtile(Wk_h[gperm]).astype(np.float16)
    wv = wtile(Wv_h).astype(np.float16)
    wo_t = np.ascontiguousarray(Wo_cols.T)  # [DOUT, D]
    wo = np.ascontiguousarray(
        wo_t.reshape(P, 128, D).transpose(1, 0, 2).reshape(128, P * D)).astype(np.float16)

    cos_t, sin_t = rope_tables(S, DH, cfg.THETA)  # [S, 32]
    # grouped rows: r in [0,128): head=r//64, pos=r%64: i=pos%32, sign=-1 if pos<32 else +1
    pos = np.arange(128) % 64
    i = pos % 32
    sign = np.where(pos < 32, -1.0, 1.0).astype(np.float32)
    cos_g = np.ascontiguousarray(cos_t.T[i]).astype(np.float16)       # [128, S]
    sin_g = np.ascontiguousarray(sin_t.T[i] * sign[:, None]).astype(np.float16)

    r = np.arange(128)
    msk01 = np.where(r[None, :] >= r[:, None], 1.0, 0.0).astype(np.float16)

    return {
        "xT": np.ascontiguousarray(x_b.T).astype(np.float16),
        "wq": wq, "wk": wk, "wv": wv, "wo": wo,
        "cos": cos_g, "sin": sin_g,
        "msk": msk01,
    }


# =========================================================================
# public entry point
# =========================================================================

_CACHE = {}


def kernel(x, Wq, Wk, Wv, Wo, lambdas=None, trace=False):
    from concourse.bass_utils import run_bass_kernel_spmd

    if not _CACHE.get("patched"):
        apply()
        _CACHE["patched"] = True
    x = np.asarray(x, dtype=np.float32)
    Wq = np.asarray(Wq, dtype=np.float32)
    Wk = np.asarray(Wk, dtype=np.float32)
    Wv = np.asarray(Wv, dtype=np.float32)
    Wo = np.asarray(Wo, dtype=np.float32)
    cfg = Cfg()
    if "nc" not in _CACHE:
        _CACHE["nc"] = build_nc(cfg)
    nc = _CACHE["nc"]
    in_maps = []
    for core in range(8):
        b, half = core // 2, core % 2
        sl = slice(half * cfg.DOUT, (half + 1) * cfg.DOUT)
        in_maps.append(prep_core_inputs(cfg, x[b], Wq[sl], Wk[sl], Wv[sl], Wo[:, sl]))
    res = run_bass_kernel_spmd(nc, in_maps, list(range(8)), trace=trace)
    outs = res.results
    out = np.stack(
        [(outs[2 * b]["outT"] + outs[2 * b + 1]["outT"]).T for b in range(4)]
    ).astype(np.float32)
    if trace:
        return out, res
    return out
